# revision 16
# baseline (speedup 1.0000x reference)
"""Trainium2 kernel for nn_DifferentiableBiquad.

Cascade of 4 biquad IIR filters over (B=32, L=524288), f32.

The pole radii are sigmoid(logit)*0.999 (actual inputs give r_max ~
0.71), so the cascade impulse response decays below 1e-5 of its peak
within ~30 lags. The IIR is computed as a truncated FIR via banded
block-Toeplitz matmuls on the TensorEngine, in bf16 (the 2e-2 rel-err
budget dwarfs bf16 quantization at ~2e-3):

  - x is cast to bf16 and transposed on the host into xin[r] =
    [128, 1 + L/128]: partition m holds sample m of every 128-block,
    column 0 is a zero block (row-start history). HBM runs are 8KB per
    partition, and no on-device transposes or boundary fixups are
    needed: every stationary operand is a stride-8 column view.
  - Per 128 x 1024 PSUM tile (chunk p = 1024 output samples): one
    NC1-wide tail matmul (previous-block history taps, Hb columns
    128:128+NC1) plus eight banded matmuls with stationary = blocks
    8p+dlt (X columns base+1+dlt, stride 8) against Hb[:, 0:128+NC1],
    where Hb[m, n] = h[n - m]. The dlt=3 matmul is split at column 512
    so no matmul write crosses a 2KB PSUM bank; each bank's first
    matmul carries start=True, per-element has_written bits turn later
    first touches into stores.
  - PSUM f32 -> SBUF bf16 evictions alternate between the DVE and
    Activation engines; output DMAs (2KB runs) alternate between the
    sync and scalar HWDGE rings, with inputs also on sync.

Batch dim (32) is sharded over 8 NeuronCores (4 rows each); rows are
independent (zero initial state == zero column 0).
"""
import math

import numpy as np

NUM_FILTERS = 4
MAX_RADIUS = 0.999
B, L = 32, 524288
N_CORES = 8
ROWS_PER_CORE = B // N_CORES
NBLK = 128                    # block size == SBUF partitions
W = 1024                      # output samples per PSUM tile partition
NBLOCKS = L // NBLK           # 4096
NGROUPS = L // (NBLK * W)     # 4 psum-tile groups per row
TAP_THR = 1e-5                # impulse-response truncation threshold


# ---------------------------------------------------------------- host math
def _coeffs_f32(log_radius, raw_angle):
    lr = np.asarray(log_radius, np.float32)
    ra = np.asarray(raw_angle, np.float32)
    radius = (np.float32(1.0) / (np.float32(1.0) + np.exp(-lr, dtype=np.float32))) * np.float32(MAX_RADIUS)
    angle = (np.float32(1.0) / (np.float32(1.0) + np.exp(-ra, dtype=np.float32))) * np.float32(math.pi)
    a1 = np.float32(-2.0) * radius * np.cos(angle, dtype=np.float32)
    a2 = radius * radius
    return a1.astype(np.float32), a2.astype(np.float32)


def _impulse_response(a1, a2, b0, b1, b2, T=256):
    h = np.zeros(T, np.float64)
    h[0] = 1.0
    for f in range(NUM_FILTERS):
        s1 = s2 = 0.0
        out = np.zeros(T, np.float64)
        for n in range(T):
            xn = h[n]
            yn = float(b0[f]) * xn + s1
            s1 = float(b1[f]) * xn - float(a1[f]) * yn + s2
            s2 = float(b2[f]) * xn - float(a2[f]) * yn
            out[n] = yn
        h = out
    return h


def _build_hb(inputs):
    a1, a2 = _coeffs_f32(inputs["log_radius"], inputs["raw_angle"])
    h = _impulse_response(
        a1, a2,
        np.asarray(inputs["b0"], np.float64),
        np.asarray(inputs["b1"], np.float64),
        np.asarray(inputs["b2"], np.float64),
    )
    hmax = np.abs(h).max()
    tap_max = int(np.max(np.nonzero(np.abs(h) > TAP_THR * hmax)))
    assert tap_max <= 127, (
        f"impulse response too long for single-shift kernel (tap_max={tap_max})"
    )
    NC1 = max(1, min(128, tap_max))
    n_idx = np.arange(NBLK)
    m_idx = np.arange(NBLK)
    lag0 = n_idx[None, :] - m_idx[:, None]           # [m, n]
    H0T = np.where((lag0 >= 0) & (lag0 <= tap_max), h[np.clip(lag0, 0, 255)], 0.0)
    lag1 = 128 + n_idx[None, :NC1] - m_idx[:, None]  # [m, n]
    H1T = np.where((lag1 >= 1) & (lag1 <= tap_max), h[np.clip(lag1, 0, 255)], 0.0)
    return np.concatenate([H0T, H1T], axis=1)        # [128, 128+NC1]


# ---------------------------------------------------------------- program
_PROGRAM_CACHE = {}


def build_program(n_rows, length, NC1):
    import concourse.mybir as mybir
    from concourse import bacc
    from concourse.tile import TileContext

    f32 = mybir.dt.float32
    bf16 = mybir.dt.bfloat16
    ncols = length // NBLK + 1           # zero col + one col per block
    ngroups = length // (NBLK * W)       # psum tiles per row
    gcols = W // NBLK                    # 8 blocks per chunk
    pad = gcols - 1                      # stride-8 view bound slack

    nc = bacc.Bacc("TRN2", target_bir_lowering=False, debug=False,
                   enable_asserts=False, num_devices=N_CORES)
    xin = nc.dram_tensor("xin", [n_rows, NBLK, ncols - 1], bf16, kind="ExternalInput")
    hb = nc.dram_tensor("hb", [NBLK, NBLK + NC1], bf16, kind="ExternalInput")
    yout = nc.dram_tensor("yout", [n_rows, length], bf16, kind="ExternalOutput")

    # Pair layout: yout_p[r, q] has dims [p, G, c] matching a [128, 2, W]
    # stage pair for groups (2q, 2q+1).
    yout_p = yout.ap().rearrange("r (q G p c) -> r q p G c", G=2, p=NBLK, c=W)

    with TileContext(nc) as tc:
        with (
            tc.tile_pool(name="const", bufs=1) as cpool,
            tc.tile_pool(name="xrow", bufs=4) as xpool,
            tc.tile_pool(name="stage", bufs=5) as spool,
            tc.tile_pool(name="py", bufs=4, space="PSUM") as pypool,
        ):
            hb_sb = cpool.tile([NBLK, NBLK + NC1], bf16, tag="hb")
            nc.scalar.dma_start(out=hb_sb[:], in_=hb.ap())

            # All input DMAs up front (all rows resident) on the sync
            # ring, which carries ONLY input: output descriptors behind
            # 4MB of queued input would stall the whole pipeline (ring is
            # FIFO). Zero history column via memset so every HBM run is
            # an aligned 2048B (the fastest measured packet size).
            xtiles = []
            for r in range(n_rows):
                X = xpool.tile([NBLK, ncols + pad], bf16, tag="x")
                nc.gpsimd.memset(X[:, 0:1], 0.0)
                for c0 in range(0, ncols - 1, W):
                    nc.sync.dma_start(
                        out=X[:, 1 + c0:1 + c0 + W],
                        in_=xin.ap()[r][:, c0:c0 + W],
                    )
                xtiles.append(X)

            for r in range(n_rows):
                X = xtiles[r]
                for g in range(ngroups):
                    base = g * W

                    def stat(col0):
                        # [128, 128] stationary: X columns col0 + 8*p
                        return X[:, col0:col0 + W].rearrange(
                            "m (c e) -> m c e", e=gcols
                        )[:, :, 0]

                    py = pypool.tile([NBLK, W], f32, tag="py")
                    # Tail: previous-block history taps into [0, NC1).
                    nc.tensor.matmul(
                        py[:, 0:NC1], stat(base),
                        hb_sb[:, NBLK:NBLK + NC1],
                        start=True, stop=False, skip_group_check=True,
                    )
                    for dlt in range(gcols):
                        lo = dlt * NBLK
                        hi = min(W, lo + NBLK + NC1)
                        st = stat(base + 1 + dlt)
                        if lo < 512 and hi > 512:
                            # Split at the PSUM bank boundary; the upper
                            # piece is bank 1's first write.
                            nc.tensor.matmul(
                                py[:, lo:512], st, hb_sb[:, 0:512 - lo],
                                start=False, stop=True, skip_group_check=True,
                            )
                            nc.tensor.matmul(
                                py[:, 512:hi], st, hb_sb[:, 512 - lo:hi - lo],
                                start=True, stop=False, skip_group_check=True,
                            )
                        else:
                            nc.tensor.matmul(
                                py[:, lo:hi], st, hb_sb[:, 0:hi - lo],
                                start=False, stop=(dlt == gcols - 1),
                                skip_group_check=True,
                            )

                    # Alternate whole-group evicts between the two
                    # PSUM-capable engines; pair two groups per output DMA
                    # (fewer posts -> less DMA-sem-pool cross-blocking),
                    # posts alternating between the two HWDGE rings.
                    if g % 2 == 0:
                        spair = spool.tile([NBLK, 2, W], bf16, tag="stage")
                        nc.vector.tensor_copy(out=spair[:, 0], in_=py[:])
                    else:
                        nc.scalar.copy(out=spair[:, 1], in_=py[:])
                        # All output posts on the scalar ring: the sync
                        # ring stays input-only, so no output descriptor
                        # ever queues ahead of input in a FIFO.
                        nc.scalar.dma_start(
                            out=yout_p[r, g // 2], in_=spair[:]
                        )
    nc.compile()
    return nc


def _get_program(n_rows, length, NC1):
    key = (n_rows, length, NC1)
    if key not in _PROGRAM_CACHE:
        _PROGRAM_CACHE[key] = build_program(*key)
    return _PROGRAM_CACHE[key]


# ---------------------------------------------------------------- entry
def _run(inputs, trace=False):
    import ml_dtypes
    from concourse.bass_utils import run_bass_kernel_spmd

    bf16 = ml_dtypes.bfloat16
    x = np.asarray(inputs["x"], np.float32)
    assert x.shape == (B, L)
    Hb = _build_hb(inputs).astype(bf16)
    NC1 = Hb.shape[1] - NBLK

    # Host-side shard layout: bf16, per-row transpose to [128, nblocks].
    xt = np.ascontiguousarray(
        x.astype(bf16).reshape(B, NBLOCKS, NBLK).swapaxes(1, 2)
    )

    nc = _get_program(ROWS_PER_CORE, L, NC1)
    xs = xt.reshape(N_CORES, ROWS_PER_CORE, NBLK, NBLOCKS)
    in_maps = [{"xin": xs[c], "hb": Hb} for c in range(N_CORES)]
    res = run_bass_kernel_spmd(nc, in_maps, core_ids=list(range(N_CORES)),
                               trace=trace)
    y = np.concatenate(
        [np.asarray(res.results[c]["yout"]) for c in range(N_CORES)], axis=0
    ).astype(np.float32).reshape(B, L)
    return y, res


def kernel(x, log_radius, raw_angle, b0, b1, b2):
    y, _ = _run(dict(x=x, log_radius=log_radius, raw_angle=raw_angle,
                     b0=b0, b1=b1, b2=b2))
    return y


# revision 17
# speedup vs baseline: 1.0415x; 1.0415x over previous
"""Trainium2 kernel for nn_DifferentiableBiquad.

Cascade of 4 biquad IIR filters over (B=32, L=524288), f32.

The pole radii are sigmoid(logit)*0.999 (actual inputs give r_max ~
0.71), so the cascade impulse response decays below 1e-5 of its peak
within ~30 lags. The IIR is computed as a truncated FIR via banded
block-Toeplitz matmuls on the TensorEngine, in bf16 (the 2e-2 rel-err
budget dwarfs bf16 quantization at ~2e-3):

  - x is cast to bf16 and transposed on the host into xin[r] =
    [128, 1 + L/128]: partition m holds sample m of every 128-block,
    column 0 is a zero block (row-start history). HBM runs are 8KB per
    partition, and no on-device transposes or boundary fixups are
    needed: every stationary operand is a stride-8 column view.
  - Per 128 x 1024 PSUM tile (chunk p = 1024 output samples): one
    NC1-wide tail matmul (previous-block history taps, Hb columns
    128:128+NC1) plus eight banded matmuls with stationary = blocks
    8p+dlt (X columns base+1+dlt, stride 8) against Hb[:, 0:128+NC1],
    where Hb[m, n] = h[n - m]. The dlt=3 matmul is split at column 512
    so no matmul write crosses a 2KB PSUM bank; each bank's first
    matmul carries start=True, per-element has_written bits turn later
    first touches into stores.
  - PSUM f32 -> SBUF bf16 evictions alternate between the DVE and
    Activation engines; output DMAs (2KB runs) alternate between the
    sync and scalar HWDGE rings, with inputs also on sync.

Batch dim (32) is sharded over 8 NeuronCores (4 rows each); rows are
independent (zero initial state == zero column 0).
"""
import math

import numpy as np

NUM_FILTERS = 4
MAX_RADIUS = 0.999
B, L = 32, 524288
N_CORES = 8
ROWS_PER_CORE = B // N_CORES
NBLK = 128                    # block size == SBUF partitions
W = 1024                      # output samples per PSUM tile partition
NBLOCKS = L // NBLK           # 4096
NGROUPS = L // (NBLK * W)     # 4 psum-tile groups per row
TAP_THR = 1e-5                # impulse-response truncation threshold


# ---------------------------------------------------------------- host math
def _coeffs_f32(log_radius, raw_angle):
    lr = np.asarray(log_radius, np.float32)
    ra = np.asarray(raw_angle, np.float32)
    radius = (np.float32(1.0) / (np.float32(1.0) + np.exp(-lr, dtype=np.float32))) * np.float32(MAX_RADIUS)
    angle = (np.float32(1.0) / (np.float32(1.0) + np.exp(-ra, dtype=np.float32))) * np.float32(math.pi)
    a1 = np.float32(-2.0) * radius * np.cos(angle, dtype=np.float32)
    a2 = radius * radius
    return a1.astype(np.float32), a2.astype(np.float32)


def _impulse_response(a1, a2, b0, b1, b2, T=256):
    h = np.zeros(T, np.float64)
    h[0] = 1.0
    for f in range(NUM_FILTERS):
        s1 = s2 = 0.0
        out = np.zeros(T, np.float64)
        for n in range(T):
            xn = h[n]
            yn = float(b0[f]) * xn + s1
            s1 = float(b1[f]) * xn - float(a1[f]) * yn + s2
            s2 = float(b2[f]) * xn - float(a2[f]) * yn
            out[n] = yn
        h = out
    return h


def _build_hb(inputs):
    a1, a2 = _coeffs_f32(inputs["log_radius"], inputs["raw_angle"])
    h = _impulse_response(
        a1, a2,
        np.asarray(inputs["b0"], np.float64),
        np.asarray(inputs["b1"], np.float64),
        np.asarray(inputs["b2"], np.float64),
    )
    hmax = np.abs(h).max()
    tap_max = int(np.max(np.nonzero(np.abs(h) > TAP_THR * hmax)))
    assert tap_max <= 127, (
        f"impulse response too long for single-shift kernel (tap_max={tap_max})"
    )
    NC1 = max(1, min(128, tap_max))
    n_idx = np.arange(NBLK)
    m_idx = np.arange(NBLK)
    lag0 = n_idx[None, :] - m_idx[:, None]           # [m, n]
    H0T = np.where((lag0 >= 0) & (lag0 <= tap_max), h[np.clip(lag0, 0, 255)], 0.0)
    lag1 = 128 + n_idx[None, :NC1] - m_idx[:, None]  # [m, n]
    H1T = np.where((lag1 >= 1) & (lag1 <= tap_max), h[np.clip(lag1, 0, 255)], 0.0)
    return np.concatenate([H0T, H1T], axis=1)        # [128, 128+NC1]


# ---------------------------------------------------------------- program
_PROGRAM_CACHE = {}


def build_program(n_rows, length, NC1):
    import concourse.mybir as mybir
    from concourse import bacc
    from concourse.tile import TileContext

    f32 = mybir.dt.float32
    bf16 = mybir.dt.bfloat16
    ncols = length // NBLK + 1           # zero col + one col per block
    ngroups = length // (NBLK * W)       # psum tiles per row
    gcols = W // NBLK                    # 8 blocks per chunk
    pad = gcols - 1                      # stride-8 view bound slack

    nc = bacc.Bacc("TRN2", target_bir_lowering=False, debug=False,
                   enable_asserts=False, num_devices=N_CORES)
    xin = nc.dram_tensor("xin", [n_rows, NBLK, ncols - 1], bf16, kind="ExternalInput")
    hb = nc.dram_tensor("hb", [NBLK, NBLK + NC1], bf16, kind="ExternalInput")
    yout = nc.dram_tensor("yout", [n_rows, length], bf16, kind="ExternalOutput")

    # Pair layout: yout_p[r, q] has dims [p, G, c] matching a [128, 2, W]
    # stage pair for groups (2q, 2q+1).
    yout_p = yout.ap().rearrange("r (q G p c) -> r q p G c", G=2, p=NBLK, c=W)

    with TileContext(nc) as tc:
        with (
            tc.tile_pool(name="const", bufs=1) as cpool,
            tc.tile_pool(name="xrow", bufs=4) as xpool,
            tc.tile_pool(name="stage", bufs=4) as spool,
            tc.tile_pool(name="py", bufs=4, space="PSUM") as pypool,
        ):
            hb_sb = cpool.tile([NBLK, NBLK + NC1], bf16, tag="hb")
            nc.scalar.dma_start(out=hb_sb[:], in_=hb.ap())

            # All input DMAs up front (all rows resident) on the sync
            # ring, which carries ONLY input: output descriptors behind
            # 4MB of queued input would stall the whole pipeline (ring is
            # FIFO). Zero history column via memset so every HBM run is
            # an aligned 2048B (the fastest measured packet size).
            xtiles = []
            for r in range(n_rows):
                X = xpool.tile([NBLK, ncols + pad], bf16, tag="x")
                nc.gpsimd.memset(X[:, 0:1], 0.0)
                for c0 in range(0, ncols - 1, W):
                    nc.sync.dma_start(
                        out=X[:, 1 + c0:1 + c0 + W],
                        in_=xin.ap()[r][:, c0:c0 + W],
                    )
                xtiles.append(X)

            for r in range(n_rows):
                X = xtiles[r]
                for g in range(ngroups):
                    base = g * W

                    def stat(col0):
                        # [128, 128] stationary: X columns col0 + 8*p
                        return X[:, col0:col0 + W].rearrange(
                            "m (c e) -> m c e", e=gcols
                        )[:, :, 0]

                    py = pypool.tile([NBLK, W], f32, tag="py")
                    # Tail: previous-block history taps into [0, NC1).
                    nc.tensor.matmul(
                        py[:, 0:NC1], stat(base),
                        hb_sb[:, NBLK:NBLK + NC1],
                        start=True, stop=False, skip_group_check=True,
                    )
                    for dlt in range(gcols):
                        lo = dlt * NBLK
                        hi = min(W, lo + NBLK + NC1)
                        st = stat(base + 1 + dlt)
                        if lo < 512 and hi > 512:
                            # Split at the PSUM bank boundary; the upper
                            # piece is bank 1's first write.
                            nc.tensor.matmul(
                                py[:, lo:512], st, hb_sb[:, 0:512 - lo],
                                start=False, stop=True, skip_group_check=True,
                            )
                            nc.tensor.matmul(
                                py[:, 512:hi], st, hb_sb[:, 512 - lo:hi - lo],
                                start=True, stop=False, skip_group_check=True,
                            )
                        else:
                            nc.tensor.matmul(
                                py[:, lo:hi], st, hb_sb[:, 0:hi - lo],
                                start=False, stop=(dlt == gcols - 1),
                                skip_group_check=True,
                            )

                    # Alternate whole-group evicts between the two
                    # PSUM-capable engines; pair two groups per output DMA
                    # (fewer posts -> less DMA-sem-pool cross-blocking),
                    # posts alternating between the two HWDGE rings.
                    if g % 2 == 0:
                        spair = spool.tile([NBLK, 2, W], bf16, tag="stage")
                        nc.vector.tensor_copy(out=spair[:, 0], in_=py[:])
                    else:
                        nc.scalar.copy(out=spair[:, 1], in_=py[:])
                        # All output posts on the scalar ring: the sync
                        # ring stays input-only, so no output descriptor
                        # ever queues ahead of input in a FIFO.
                        nc.scalar.dma_start(
                            out=yout_p[r, g // 2], in_=spair[:]
                        )
    nc.compile()
    return nc


def _get_program(n_rows, length, NC1):
    key = (n_rows, length, NC1)
    if key not in _PROGRAM_CACHE:
        _PROGRAM_CACHE[key] = build_program(*key)
    return _PROGRAM_CACHE[key]


# ---------------------------------------------------------------- entry
def _run(inputs, trace=False):
    import ml_dtypes
    from concourse.bass_utils import run_bass_kernel_spmd

    bf16 = ml_dtypes.bfloat16
    x = np.asarray(inputs["x"], np.float32)
    assert x.shape == (B, L)
    Hb = _build_hb(inputs).astype(bf16)
    NC1 = Hb.shape[1] - NBLK

    # Host-side shard layout: bf16, per-row transpose to [128, nblocks].
    xt = np.ascontiguousarray(
        x.astype(bf16).reshape(B, NBLOCKS, NBLK).swapaxes(1, 2)
    )

    nc = _get_program(ROWS_PER_CORE, L, NC1)
    xs = xt.reshape(N_CORES, ROWS_PER_CORE, NBLK, NBLOCKS)
    in_maps = [{"xin": xs[c], "hb": Hb} for c in range(N_CORES)]
    res = run_bass_kernel_spmd(nc, in_maps, core_ids=list(range(N_CORES)),
                               trace=trace)
    y = np.concatenate(
        [np.asarray(res.results[c]["yout"]) for c in range(N_CORES)], axis=0
    ).astype(np.float32).reshape(B, L)
    return y, res


def kernel(x, log_radius, raw_angle, b0, b1, b2):
    y, _ = _run(dict(x=x, log_radius=log_radius, raw_angle=raw_angle,
                     b0=b0, b1=b1, b2=b2))
    return y


# revision 20
# speedup vs baseline: 1.1390x; 1.0935x over previous
"""Trainium2 kernel for nn_DifferentiableBiquad.

Cascade of 4 biquad IIR filters over (B=32, L=524288), f32.

The pole radii are sigmoid(logit)*0.999 (actual inputs give r_max ~
0.71), so the cascade impulse response decays below 1e-5 of its peak
within ~30 lags. The IIR is computed as a truncated FIR via banded
block-Toeplitz matmuls on the TensorEngine, in bf16 (the 2e-2 rel-err
budget dwarfs bf16 quantization at ~2e-3):

  - x is cast to bf16 and transposed on the host into xin[r] =
    [128, 1 + L/128]: partition m holds sample m of every 128-block,
    column 0 is a zero block (row-start history). HBM runs are 8KB per
    partition, and no on-device transposes or boundary fixups are
    needed: every stationary operand is a stride-8 column view.
  - Per 128 x 1024 PSUM tile (chunk p = 1024 output samples): one
    NC1-wide tail matmul (previous-block history taps, Hb columns
    128:128+NC1) plus eight banded matmuls with stationary = blocks
    8p+dlt (X columns base+1+dlt, stride 8) against Hb[:, 0:128+NC1],
    where Hb[m, n] = h[n - m]. The dlt=3 matmul is split at column 512
    so no matmul write crosses a 2KB PSUM bank; each bank's first
    matmul carries start=True, per-element has_written bits turn later
    first touches into stores.
  - PSUM f32 -> SBUF bf16 evictions alternate between the DVE and
    Activation engines into [128, 2, 1024] stage pairs; one output DMA
    (2KB runs) per pair on the scalar HWDGE ring. The sync ring carries
    ONLY input (2048B aligned packets, the fastest measured size): the
    rings are FIFO, so an evict-dependent output descriptor queued
    ahead of input would stall the whole pipeline. The stage pool depth
    (5 pairs) deliberately throttles how far output posts can run ahead,
    keeping the input stream fed first.

Batch dim (32) is sharded over 8 NeuronCores (4 rows each); rows are
independent (zero initial state == zero column 0).
"""
import math

import numpy as np

NUM_FILTERS = 4
MAX_RADIUS = 0.999
B, L = 32, 524288
N_CORES = 8
ROWS_PER_CORE = B // N_CORES
NBLK = 128                    # block size == SBUF partitions
W = 1024                      # output samples per PSUM tile partition
NBLOCKS = L // NBLK           # 4096
NGROUPS = L // (NBLK * W)     # 4 psum-tile groups per row
TAP_THR = 1e-5                # impulse-response truncation threshold


# ---------------------------------------------------------------- host math
def _coeffs_f32(log_radius, raw_angle):
    lr = np.asarray(log_radius, np.float32)
    ra = np.asarray(raw_angle, np.float32)
    radius = (np.float32(1.0) / (np.float32(1.0) + np.exp(-lr, dtype=np.float32))) * np.float32(MAX_RADIUS)
    angle = (np.float32(1.0) / (np.float32(1.0) + np.exp(-ra, dtype=np.float32))) * np.float32(math.pi)
    a1 = np.float32(-2.0) * radius * np.cos(angle, dtype=np.float32)
    a2 = radius * radius
    return a1.astype(np.float32), a2.astype(np.float32)


def _impulse_response(a1, a2, b0, b1, b2, T=256):
    h = np.zeros(T, np.float64)
    h[0] = 1.0
    for f in range(NUM_FILTERS):
        s1 = s2 = 0.0
        out = np.zeros(T, np.float64)
        for n in range(T):
            xn = h[n]
            yn = float(b0[f]) * xn + s1
            s1 = float(b1[f]) * xn - float(a1[f]) * yn + s2
            s2 = float(b2[f]) * xn - float(a2[f]) * yn
            out[n] = yn
        h = out
    return h


def _build_hb(inputs):
    a1, a2 = _coeffs_f32(inputs["log_radius"], inputs["raw_angle"])
    h = _impulse_response(
        a1, a2,
        np.asarray(inputs["b0"], np.float64),
        np.asarray(inputs["b1"], np.float64),
        np.asarray(inputs["b2"], np.float64),
    )
    hmax = np.abs(h).max()
    tap_max = int(np.max(np.nonzero(np.abs(h) > TAP_THR * hmax)))
    assert tap_max <= 127, (
        f"impulse response too long for single-shift kernel (tap_max={tap_max})"
    )
    NC1 = max(1, min(128, tap_max))
    n_idx = np.arange(NBLK)
    m_idx = np.arange(NBLK)
    lag0 = n_idx[None, :] - m_idx[:, None]           # [m, n]
    H0T = np.where((lag0 >= 0) & (lag0 <= tap_max), h[np.clip(lag0, 0, 255)], 0.0)
    lag1 = 128 + n_idx[None, :NC1] - m_idx[:, None]  # [m, n]
    H1T = np.where((lag1 >= 1) & (lag1 <= tap_max), h[np.clip(lag1, 0, 255)], 0.0)
    return np.concatenate([H0T, H1T], axis=1)        # [128, 128+NC1]


# ---------------------------------------------------------------- program
_PROGRAM_CACHE = {}


def build_program(n_rows, length, NC1):
    import concourse.mybir as mybir
    from concourse import bacc
    from concourse.tile import TileContext

    f32 = mybir.dt.float32
    bf16 = mybir.dt.bfloat16
    ncols = length // NBLK + 1           # zero col + one col per block
    ngroups = length // (NBLK * W)       # psum tiles per row
    gcols = W // NBLK                    # 8 blocks per chunk
    pad = gcols - 1                      # stride-8 view bound slack

    nc = bacc.Bacc("TRN2", target_bir_lowering=False, debug=False,
                   enable_asserts=False, num_devices=N_CORES)
    xin = nc.dram_tensor("xin", [n_rows, NBLK, ncols - 1], bf16, kind="ExternalInput")
    hb = nc.dram_tensor("hb", [NBLK, NBLK + NC1], bf16, kind="ExternalInput")
    yout = nc.dram_tensor("yout", [n_rows, length], bf16, kind="ExternalOutput")

    # Pair layout: yout_p[r, q] has dims [p, G, c] matching a [128, 2, W]
    # stage pair for groups (2q, 2q+1).
    yout_p = yout.ap().rearrange("r (q G p c) -> r q p G c", G=2, p=NBLK, c=W)

    with TileContext(nc) as tc:
        with (
            tc.tile_pool(name="const", bufs=1) as cpool,
            tc.tile_pool(name="xrow", bufs=4) as xpool,
            tc.tile_pool(name="stage", bufs=5) as spool,
            tc.tile_pool(name="py", bufs=4, space="PSUM") as pypool,
        ):
            hb_sb = cpool.tile([NBLK, NBLK + NC1], bf16, tag="hb")
            nc.scalar.dma_start(out=hb_sb[:], in_=hb.ap())

            # All input DMAs up front (all rows resident) on the sync
            # ring, which carries ONLY input: output descriptors behind
            # 4MB of queued input would stall the whole pipeline (ring is
            # FIFO). Zero history column via memset so every HBM run is
            # an aligned 2048B (the fastest measured packet size).
            xtiles = []
            for r in range(n_rows):
                X = xpool.tile([NBLK, ncols + pad], bf16, tag="x")
                nc.gpsimd.memset(X[:, 0:1], 0.0)
                for c0 in range(0, ncols - 1, W):
                    nc.sync.dma_start(
                        out=X[:, 1 + c0:1 + c0 + W],
                        in_=xin.ap()[r][:, c0:c0 + W],
                    )
                xtiles.append(X)

            for r in range(n_rows):
                X = xtiles[r]
                for g in range(ngroups):
                    base = g * W

                    def stat(col0):
                        # [128, 128] stationary: X columns col0 + 8*p
                        return X[:, col0:col0 + W].rearrange(
                            "m (c e) -> m c e", e=gcols
                        )[:, :, 0]

                    py = pypool.tile([NBLK, W], f32, tag="py")
                    # Tail: previous-block history taps into [0, NC1).
                    nc.tensor.matmul(
                        py[:, 0:NC1], stat(base),
                        hb_sb[:, NBLK:NBLK + NC1],
                        start=True, stop=False, skip_group_check=True,
                    )
                    for dlt in range(gcols):
                        lo = dlt * NBLK
                        hi = min(W, lo + NBLK + NC1)
                        st = stat(base + 1 + dlt)
                        if lo < 512 and hi > 512:
                            # Split at the PSUM bank boundary; the upper
                            # piece is bank 1's first write.
                            nc.tensor.matmul(
                                py[:, lo:512], st, hb_sb[:, 0:512 - lo],
                                start=False, stop=True, skip_group_check=True,
                            )
                            nc.tensor.matmul(
                                py[:, 512:hi], st, hb_sb[:, 512 - lo:hi - lo],
                                start=True, stop=False, skip_group_check=True,
                            )
                        else:
                            nc.tensor.matmul(
                                py[:, lo:hi], st, hb_sb[:, 0:hi - lo],
                                start=False, stop=(dlt == gcols - 1),
                                skip_group_check=True,
                            )

                    # Alternate whole-group evicts between the two
                    # PSUM-capable engines; pair two groups per output DMA
                    # (fewer posts -> less DMA-sem-pool cross-blocking).
                    if g % 2 == 0:
                        spair = spool.tile([NBLK, 2, W], bf16, tag="stage")
                        nc.vector.tensor_copy(out=spair[:, 0], in_=py[:])
                    else:
                        nc.scalar.copy(out=spair[:, 1], in_=py[:])
                        # All output posts on the scalar ring: the sync
                        # ring stays input-only, so no output descriptor
                        # ever queues ahead of input in a FIFO.
                        nc.scalar.dma_start(
                            out=yout_p[r, g // 2], in_=spair[:]
                        )
    nc.compile()
    return nc


def _get_program(n_rows, length, NC1):
    key = (n_rows, length, NC1)
    if key not in _PROGRAM_CACHE:
        _PROGRAM_CACHE[key] = build_program(*key)
    return _PROGRAM_CACHE[key]


# ---------------------------------------------------------------- entry
def _run(inputs, trace=False):
    import ml_dtypes
    from concourse.bass_utils import run_bass_kernel_spmd

    bf16 = ml_dtypes.bfloat16
    x = np.asarray(inputs["x"], np.float32)
    assert x.shape == (B, L)
    Hb = _build_hb(inputs).astype(bf16)
    NC1 = Hb.shape[1] - NBLK

    # Host-side shard layout: bf16, per-row transpose to [128, nblocks].
    xt = np.ascontiguousarray(
        x.astype(bf16).reshape(B, NBLOCKS, NBLK).swapaxes(1, 2)
    )

    nc = _get_program(ROWS_PER_CORE, L, NC1)
    xs = xt.reshape(N_CORES, ROWS_PER_CORE, NBLK, NBLOCKS)
    in_maps = [{"xin": xs[c], "hb": Hb} for c in range(N_CORES)]
    res = run_bass_kernel_spmd(nc, in_maps, core_ids=list(range(N_CORES)),
                               trace=trace)
    y = np.concatenate(
        [np.asarray(res.results[c]["yout"]) for c in range(N_CORES)], axis=0
    ).astype(np.float32).reshape(B, L)
    return y, res


def kernel(x, log_radius, raw_angle, b0, b1, b2):
    y, _ = _run(dict(x=x, log_radius=log_radius, raw_angle=raw_angle,
                     b0=b0, b1=b1, b2=b2))
    return y
